# revision 20
# baseline (speedup 1.0000x reference)
"""CTC loss (keras ctc_batch_cost semantics, blank=C-1) on 8 TRN2 NeuronCores.

Strategy (v2)
-------------
Data-parallel: 1024 examples sharded 128 per core. Per core:

1. Supply pipeline (tensor/gpsimd/scalar/sync queues, overlapped with DP):
   y chunks DMA'd in (e,tau) layout -> gpsimd ap_gather pulls the 48 label
   classes + blank per timestep -> fp32->bf16 convert on scalar -> bf16
   DRAM-bounce transpose to example-major [128, t, 49] chunk tiles.
2. Per chunk: rbl = 1/blank (bf16), sum-ln(blank) accumulated on scalar, and
   a per-step "labs" table on vector: labs[slot] = [lab*rbl | lab*rbl*mask].
   Bwd slots store label-reversed, time-reversed entries.
3. Blank-normalized probability-domain DP in bf16 with a masked-G (Gm)
   state, 3 DVE ops/step (all tensor_tensor, 2x bf16 mode):
     opA: Y = G + Gm_shift
     opB: [F'|X] = F + [G_shift|Y]     (fused 2-block)
     opC: [G'|Gm'] = X * [labq|labqm]  (fused 2-block)
   Phase 1: 96 forward-only steps (t=1..96) overlapping the supply stream.
   Phase 2: 79 fused forward+backward steps (fwd t=97..175, bwd t=254..176,
   backward stored flipped so both dirs share one instruction via a dir-
   stride block). Renorm every 16 steps via tensor_tensor_reduce accum.
4. Combine at the meeting point (fwd through t=175, bwd through t=176):
   one extra opA/opB on bwd gives the transition-gathered C vector; two
   reversed tensor_tensor_reduce dots against fwd F/G give the total.
   loss = -(ln(total) + sum_t ln bl_t + sum ln c_phase1 + 2 sum ln c_phase2)

State cols per direction (dir base D = 0 fwd, 256 bwd):
  D+0 zG | D+1..48 G | D+49..96 Y | D+97 zGm | D+98..145 Gm
  D+146..194 F | D+195..243 X (col 243 junk)
"""

import numpy as np

B, T, C, L = 1024, 256, 128, 48
NCORES = 8
BC = B // NCORES          # 128 examples per core
NM, NW, TW = 16, 4, 64    # 16 example-batches of 8; 4 time-chunks of 64
GIW = 16                  # int16 idx slots per batch (13 used, 32B stride)
KPT = 208                 # gather idxs per partition (16*13; 196 real)

P1 = 96                   # phase-1 fwd-only steps (t=1..P1)
N2 = 79                   # fused steps (fwd t=P1+k, bwd t=255-k, k=1..N2)
BOFF = 176                # labs slot offset of bwd slots (slot k <-> t=255-k)
NR = 16                   # renorm period (steps)
SD = 256                  # state dir stride (cols)
ZG, G0, Y0, ZGM, GM0, F0, X0 = 0, 1, 49, 97, 98, 146, 195
STW = 500                 # state tile width
# fused opC labs dir stride varies per step: ((BOFF+79-k)-(P1+k))*96

_CACHED = {}


def _host_gidx(labels_core):
    """[128, NM*GIW] int16 ap_gather indices, one example per 16-part group.

    For batch m, group g (example 8m+g), out free pos k = tt*49 + j with
    j=0 -> blank(127), j>=1 -> labels[e, j-1]; k in [0,196).
    ap_gather unwraps indices in (s p) order: value for k sits at
    (partition 16g + k%16, slot k//16).
    """
    k = np.arange(KPT)
    tt, j = k // 49, k % 49
    valid = k < 196
    lab = labels_core.reshape(NM, 8, L)
    jl = np.clip(j - 1, 0, L - 1)
    vals = np.where(j[None, None, :] == 0, 127, lab[:, :, jl])
    vals = np.where(valid[None, None, :], vals + 128 * tt[None, None, :], 0)
    gidx = np.zeros((128, NM, GIW), np.int16)
    for m in range(NM):
        for g in range(8):
            gidx[16 * g + (k % 16), m, k // 16] = vals[m, g, :]
    return gidx.reshape(128, NM * GIW)


def _host_masks(lc):
    m = np.zeros((BC, L), np.float32)
    m[:, 1:] = (lc[:, 1:] != lc[:, :-1]).astype(np.float32)
    mF = np.concatenate([m[:, 1:], np.zeros((BC, 1), np.float32)], 1)
    return mF, m


def _build_nc(debug=False, stage=99):
    from contextlib import ExitStack
    import concourse.bacc as bacc
    import concourse.tile as tile
    import concourse.mybir as mybir
    from concourse.ap import AP

    f32 = mybir.dt.float32
    bf16 = mybir.dt.bfloat16
    Alu = mybir.AluOpType
    Act = mybir.ActivationFunctionType

    nc = bacc.Bacc("TRN2", target_bir_lowering=False, debug=False)
    yD = nc.dram_tensor("y", [BC, T, C], f32, kind="ExternalInput").ap()
    gidxD = nc.dram_tensor("gidx", [128, NM * GIW], mybir.dt.int16,
                           kind="ExternalInput").ap()
    maskFD = nc.dram_tensor("maskF", [128, L], f32, kind="ExternalInput").ap()
    maskBD = nc.dram_tensor("maskB", [128, L], f32, kind="ExternalInput").ap()
    outD = nc.dram_tensor("out", [BC, 1], f32, kind="ExternalOutput").ap()
    if debug:
        dbg = {
            "dchunk0": nc.dram_tensor("dchunk0", [128, TW * 49], bf16,
                                      kind="ExternalOutput").ap(),
            "dlabs": nc.dram_tensor("dlabs", [128, 512], bf16,
                                    kind="ExternalOutput").ap(),
            "dS1": nc.dram_tensor("dS1", [128, STW], bf16,
                                  kind="ExternalOutput").ap(),
            "dSF": nc.dram_tensor("dSF", [128, STW], bf16,
                                  kind="ExternalOutput").ap(),
            "dnorm": nc.dram_tensor("dnorm", [128, 12], f32,
                                    kind="ExternalOutput").ap(),
            "dacc": nc.dram_tensor("dacc", [128, 8], f32,
                                   kind="ExternalOutput").ap(),
        }

    with tile.TileContext(nc) as tc, ExitStack() as ctx:
        cpool = ctx.enter_context(tc.tile_pool(name="const", bufs=1))
        spool = ctx.enter_context(tc.tile_pool(name="small", bufs=1))
        ypool = ctx.enter_context(tc.tile_pool(name="ystage", bufs=6))
        gpool = ctx.enter_context(tc.tile_pool(name="gout", bufs=4))
        g16pool = ctx.enter_context(tc.tile_pool(name="g16", bufs=4))
        dpool = ctx.enter_context(tc.tile_pool(name="dscr", bufs=4,
                                               space="DRAM"))

        gidxT = cpool.tile([128, NM * GIW], mybir.dt.int16)
        nc.sync.dma_start(out=gidxT[:], in_=gidxD)
        maskF32 = cpool.tile([128, L], f32)
        nc.sync.dma_start(out=maskF32[:], in_=maskFD)
        maskB32 = cpool.tile([128, L], f32)
        nc.sync.dma_start(out=maskB32[:], in_=maskBD)
        maskF16 = cpool.tile([128, L], bf16)
        nc.scalar.copy(maskF16[:], maskF32[:])
        maskB16 = cpool.tile([128, L], bf16)
        nc.scalar.copy(maskB16[:], maskB32[:])

        labsT = cpool.tile([128, 256 * 96], bf16)
        chunkT = [cpool.tile([128, TW * 49], bf16, name=f"chunk{i}")
                  for i in range(NW)]
        rblT = [cpool.tile([128, TW], bf16, name=f"rbl{i}")
                for i in range(NW)]
        lnscr = [cpool.tile([128, TW], f32, name=f"lnscr{i}")
                 for i in range(NW)]

        Sa = spool.tile([128, STW], bf16)
        Sb = spool.tile([128, STW], bf16)
        norms1 = spool.tile([128, 6], f32)
        norms2 = spool.tile([128, 4], f32)
        lnblacc = spool.tile([128, NW], f32)
        recT = spool.tile([128, 1], f32)
        scrE = spool.tile([128, 49], bf16)
        scrO = spool.tile([128, 48], bf16)
        accE = spool.tile([128, 1], f32)
        accO = spool.tile([128, 1], f32)
        tot = spool.tile([128, 1], f32)
        lnfin = spool.tile([128, 1], f32)
        lnn1 = spool.tile([128, 6], f32)
        lnn2 = spool.tile([128, 4], f32)
        acc1 = spool.tile([128, 1], f32)
        acc2 = spool.tile([128, 1], f32)
        lnblsum = spool.tile([128, 1], f32)
        tmp = spool.tile([128, 1], f32)
        lossT = spool.tile([128, 1], f32)

        nc.vector.memset(Sa[:], 0.0)
        nc.vector.memset(Sb[:], 0.0)
        if stage < 4:
            nc.vector.memset(labsT[:], 0.0)
            for _w in range(NW):
                nc.vector.memset(rblT[_w][:], 0.0)
        if stage < 99:
            nc.vector.memset(norms1[:], 1.0)
            nc.vector.memset(norms2[:], 1.0)
        nc.vector.memset(Sa[:, F0:F0 + 1], 1.0)  # fwd F_0 = 1

        # y[(m e) (w tau tt) c] -> [w m e tau (tt c)]
        yv = yD.rearrange("(m e) (w tau tt) c -> w m e tau (tt c)",
                          m=NM, e=8, w=NW, tau=16, tt=4)

        def pd(ap):
            return list(ap.ap[0])

        def vap(t_ap, coloff, dims):
            base = t_ap[:, coloff:coloff + 1]
            return AP(base.tensor, base.offset, [pd(base)] + dims)

        # ---------- supply pipeline (no vector engine) ----------
        dscr = [None] * NW

        def supply(w):
            dscr[w] = dpool.tile([128, TW * 49], bf16, name=f"dscr{w}")
            db = dscr[w][:]
            for m in range(NM):
                st = ypool.tile([128, 512], f32)
                nc.sync.dma_start(out=st[:], in_=yv[w, m])
                gob = gpool.tile([128, KPT], f32)
                nc.gpsimd.ap_gather(gob[:], st[:],
                                    gidxT[:, GIW * m:GIW * m + 13],
                                    channels=128, num_elems=512, d=1,
                                    num_idxs=KPT)
                g16 = g16pool.tile([128, KPT], bf16)
                nc.scalar.copy(g16[:], gob[:])
                dst = AP(db.tensor, db.offset + 8 * m * (TW * 49),
                         [[TW * 49, 8], [196, 16], [1, 196]])
                nc.scalar.dma_start(out=dst, in_=g16[:, 0:196])
            nc.scalar.dma_start(out=chunkT[w][:], in_=db)

        # ---------- vector-side per-chunk prep ----------
        def prep(w, fwd_rng=None, bwd_rng=None):
            ch = chunkT[w]
            blv = vap(ch, 0, [[49, TW]])
            with nc.allow_low_precision(reason="bf16 rbl validated 8e-5"):
                nc.vector.reciprocal(rblT[w][:], blv)
            if fwd_rng is not None:
                tl0, nt = fwd_rng
                s0 = TW * w + tl0  # slot = t
                nc.vector.tensor_tensor(
                    vap(labsT, s0 * 96, [[1, 48], [96, nt]]),
                    vap(ch, tl0 * 49 + 1, [[1, 48], [49, nt]]),
                    vap(rblT[w], tl0, [[0, 48], [1, nt]]),
                    Alu.mult)
                nc.vector.tensor_tensor(
                    vap(labsT, s0 * 96 + 48, [[96, nt], [1, 48]]),
                    vap(labsT, s0 * 96, [[96, nt], [1, 48]]),
                    vap(maskF16, 0, [[0, nt], [1, 48]]),
                    Alu.mult)
            if bwd_rng is not None:
                tl0, nt = bwd_rng
                s0 = BOFF + (TW * w + tl0) - 176  # slot = BOFF + t - 176
                nc.vector.tensor_tensor(
                    vap(labsT, s0 * 96, [[1, 48], [96, nt]]),
                    vap(ch, tl0 * 49 + 1, [[1, 48], [49, nt]]),
                    vap(rblT[w], tl0, [[0, 48], [1, nt]]),
                    Alu.mult)
                nc.vector.tensor_tensor(
                    vap(labsT, s0 * 96 + 48, [[96, nt], [1, 48]]),
                    vap(labsT, s0 * 96, [[96, nt], [1, 48]]),
                    vap(maskB16, 0, [[0, nt], [1, 48]]),
                    Alu.mult)

        # ---------- DP step ----------
        state = [Sa, Sb]

        def emit_step(ci, fused, slot_f, kf=0, renorm_to=None):
            cur, nxt = state[ci], state[1 - ci]
            if fused:
                span = STW
            else:
                span = 244
            if fused:
                nc.vector.tensor_tensor(
                    vap(cur, Y0, [[SD, 2], [1, 48]]),
                    vap(cur, G0, [[SD - 1, 2], [1, 48]]),
                    vap(cur, ZGM, [[SD + 1, 2], [1, 48]]),
                    Alu.add)
                nc.vector.tensor_tensor(
                    vap(nxt, F0, [[49, 2], [1, 49]]),
                    vap(cur, F0, [[0, 2], [1, 49]]),
                    vap(cur, ZG, [[49, 2], [1, 49]]),
                    Alu.add)
                nc.vector.tensor_tensor(
                    vap(nxt, SD + F0, [[49, 2], [1, 49]]),
                    vap(cur, SD + F0, [[1, 2], [1, 49]]),
                    vap(cur, SD + 0, [[49, 2], [1, 49]]),
                    Alu.add)
            else:
                nc.vector.tensor_tensor(
                    vap(cur, Y0, [[1, 48]]),
                    vap(cur, G0, [[1, 48]]),
                    vap(cur, ZGM, [[1, 48]]),
                    Alu.add)
                nc.vector.tensor_tensor(
                    vap(nxt, F0, [[49, 2], [1, 49]]),
                    vap(cur, F0, [[0, 2], [1, 49]]),
                    vap(cur, ZG, [[49, 2], [1, 49]]),
                    Alu.add)
            inX = vap(nxt, X0, [[SD, 2], [1, 48]] if fused
                      else [[0, 2], [1, 48]])
            if fused:
                dlt = ((BOFF + 79 - kf) - slot_f) * 96
                lq = vap(labsT, slot_f * 96, [[dlt, 2], [1, 48]])
                lqm = vap(labsT, slot_f * 96 + 48, [[dlt, 2], [1, 48]])
                outG = vap(nxt, G0, [[SD - 1, 2], [1, 48]])
                outGm = vap(nxt, GM0, [[SD - 1, 2], [1, 48]])
                nc.vector.tensor_tensor(outG, inX, lq, Alu.mult)
                nc.vector.tensor_tensor(outGm, inX, lqm, Alu.mult)
                if renorm_to is not None:
                    nt_, idx = renorm_to
                    nc.vector.tensor_reduce(
                        nt_[:, idx:idx + 1],
                        vap(nxt, G0, [[SD - 1, 2], [1, 48]]),
                        mybir.AxisListType.XY, Alu.add)
            else:
                labs_in1 = vap(labsT, slot_f * 96, [[48, 2], [1, 48]])
                outC = vap(nxt, G0, [[97, 2], [1, 48]])
                nc.vector.tensor_tensor(outC, inX, labs_in1, Alu.mult)
                if renorm_to is not None:
                    nt_, idx = renorm_to
                    nc.vector.tensor_reduce(
                        nt_[:, idx:idx + 1],
                        vap(nxt, G0, [[97, 2], [1, 48]]),
                        mybir.AxisListType.XY, Alu.add)
            if renorm_to is not None:
                nt_, idx = renorm_to
                nc.vector.reciprocal(recT[:], nt_[:, idx:idx + 1])
                nc.vector.tensor_scalar_mul(nxt[:, 0:span], nxt[:, 0:span],
                                            recT[:])
            return 1 - ci

        # ================= emission =================
        order = [0, 1, 3, 2]
        for w in order:
            supply(w)
        for w in order:
            blv = vap(chunkT[w], 0, [[49, TW]])
            nc.scalar.activation(lnscr[w][:], blv, Act.Ln,
                                 accum_out=lnblacc[:, w:w + 1])

        # vector stream:
        if stage >= 2 or stage == -1:
            prep(0, fwd_rng=(0, TW))
        ci = 0  # cur = Sa
        if stage >= 2:
            prep(1, fwd_rng=(0, TW))
            prep(3, bwd_rng=(0, TW))
            prep(2, fwd_rng=(0, 48), bwd_rng=(48, 16))
        # fwd init (t=0)
        if stage >= 3:
            nc.vector.tensor_scalar_add(state[ci][:, G0:G0 + 1],
                                        labsT[:, 0:1], 0.0)
            nc.vector.tensor_scalar_add(state[ci][:, GM0:GM0 + 1],
                                        labsT[:, 48:49], 0.0)
        k1 = 0
        for t in (range(1, P1 + 1) if stage >= 4 else []):
            if t == TW and stage < 2:
                prep(1, fwd_rng=(0, TW))
            rn = (norms1, k1) if t % NR == 0 else None
            ci = emit_step(ci, False, t, renorm_to=rn)
            if rn:
                k1 += 1
        if debug:
            nc.sync.dma_start(out=dbg["dchunk0"],
                              in_=chunkT[0][:])  # bf16->f32? dtype mismatch
        if stage >= 5 and stage < 2:
            prep(3, bwd_rng=(0, TW))
        # bwd init (t=255 -> slot BOFF+79): Fb_48=1, Gb_47, Gmb_47
        cur = state[ci]
        s255 = (BOFF + 79) * 96
        nc.vector.memset(cur[:, SD + 194:SD + 195], 1.0)
        nc.vector.tensor_scalar_add(cur[:, SD + 47:SD + 48],
                                    labsT[:, s255 + 47:s255 + 48], 0.0)
        nc.vector.tensor_scalar_add(cur[:, SD + 144:SD + 145],
                                    labsT[:, s255 + 48 + 47:s255 + 48 + 48],
                                    0.0)
        if debug:
            nc.sync.dma_start(out=dbg["dS1"], in_=cur[:])
        k2 = 0
        for k in (range(1, N2 + 1) if stage >= 5 else []):
            if k == 32 and stage < 2:
                prep(2, fwd_rng=(0, 48), bwd_rng=(48, 16))
            rn = (norms2, k2) if k % NR == 0 else None
            ci = emit_step(ci, True, P1 + k, kf=k, renorm_to=rn)
            if rn:
                k2 += 1
        # ---------- combine ----------
        cur, nxt = state[ci], state[1 - ci]
        if stage < 1:
            nc.vector.memset(lossT[:], 0.0)
        if stage >= 1:
            nc.vector.tensor_tensor(
                vap(cur, SD + Y0, [[1, 48]]),
                vap(cur, SD + 0, [[1, 48]]),
                vap(cur, SD + GM0, [[1, 48]]),
                Alu.add)
            nc.vector.tensor_tensor(
                vap(nxt, SD + F0, [[49, 2], [1, 49]]),
                vap(cur, SD + F0, [[1, 2], [1, 49]]),
                vap(cur, SD + 0, [[49, 2], [1, 49]]),
                Alu.add)
            nc.vector.tensor_tensor(
                scrE[:], vap(cur, F0, [[1, 49]]),
                vap(nxt, SD + F0, [[1, 49]]), Alu.mult)
            nc.vector.tensor_tensor(
                scrO[:], vap(cur, G0, [[1, 48]]),
                vap(nxt, SD + X0, [[1, 48]]), Alu.mult)
            nc.vector.tensor_reduce(accE[:], scrE[:], mybir.AxisListType.X,
                                    Alu.add)
            nc.vector.tensor_reduce(accO[:], scrO[:], mybir.AxisListType.X,
                                    Alu.add)
            nc.vector.tensor_add(tot[:], accE[:], accO[:])
            nc.scalar.activation(lnfin[:], tot[:], Act.Ln)
            nc.scalar.activation(lnn1[:], norms1[:], Act.Ln, accum_out=acc1[:])
            nc.scalar.activation(lnn2[:], norms2[:], Act.Ln,
                                 accum_out=acc2[:])
            nc.vector.tensor_reduce(lnblsum[:], lnblacc[:],
                                    mybir.AxisListType.X, Alu.add)
            nc.vector.scalar_tensor_tensor(tmp[:], acc2[:], 2.0, acc1[:],
                                           Alu.mult, Alu.add)
            nc.vector.tensor_add(tmp[:], tmp[:], lnblsum[:])
            nc.vector.tensor_add(tmp[:], tmp[:], lnfin[:])
            nc.vector.tensor_scalar_mul(lossT[:], tmp[:], -1.0)
        if debug:
            nc.sync.dma_start(out=dbg["dSF"], in_=cur[:])
            nc.sync.dma_start(out=dbg["dnorm"][:, 0:6], in_=norms1[:])
            nc.sync.dma_start(out=dbg["dnorm"][:, 6:10], in_=norms2[:])
            nc.sync.dma_start(out=dbg["dacc"][:, 0:1], in_=accE[:])
            nc.sync.dma_start(out=dbg["dacc"][:, 1:2], in_=accO[:])
            nc.sync.dma_start(out=dbg["dacc"][:, 2:3], in_=lnfin[:])
            nc.sync.dma_start(out=dbg["dacc"][:, 3:4], in_=acc1[:])
            nc.sync.dma_start(out=dbg["dacc"][:, 4:5], in_=acc2[:])
            nc.sync.dma_start(out=dbg["dacc"][:, 5:6], in_=lnblsum[:])
        nc.sync.dma_start(out=outD, in_=lossT[:])

    nc.compile()
    return nc


def _get_nc():
    if "nc" not in _CACHED:
        _CACHED["nc"] = _build_nc()
    return _CACHED["nc"]


def make_in_maps(y_pred, labels):
    y_pred = np.ascontiguousarray(np.asarray(y_pred, np.float32))
    labels = np.asarray(labels, np.int32)
    in_maps = []
    for c in range(NCORES):
        sl = slice(BC * c, BC * (c + 1))
        lc = labels[sl]
        mF, mB = _host_masks(lc)
        in_maps.append({
            "y": np.ascontiguousarray(y_pred[sl]),
            "gidx": _host_gidx(lc),
            "maskF": mF,
            "maskB": mB,
        })
    return in_maps


def kernel(y_pred, labels):
    from concourse.bass_utils import run_bass_kernel_spmd
    nc = _get_nc()
    in_maps = make_in_maps(y_pred, labels)
    res = run_bass_kernel_spmd(nc, in_maps, list(range(NCORES)))
    return np.concatenate([res.results[c]["out"] for c in range(NCORES)], 0)
